# revision 12
# baseline (speedup 1.0000x reference)
"""nn_Model_1889785610620 — dense transformer (3 enc) + 2-layer BiGRU + maxpool + FC.

Bass/Tile device kernel, data-parallel over batch across 8 TRN2 NeuronCores
(16 batch rows per core). Host does: sharding, embedding gather, weight
repacking into SBUF layouts, and final output concat. Device does everything
else, in fp32 throughout (the harness gate is pointwise rel err < 2e-2 and
some output logits are ~1e-3, so bf16 compute is insufficient).

Layouts (per core):
  - Activations feature-major: X^T stored [128 part, 4*1600], col =
    kc*1600 + b*100 + s  (kc = feature chunk of 128).
  - Attention uses the reference's contiguous view(B*NH, S, DH) head split:
    per batch the [100, 512] block viewed as [800, 64]; j = 8*s + dc,
    head h = rows [100h, 100h+100).  Processed in batch-groups of 4:
    Q/K/V produced by M=64 matmuls and scattered into "j-feature-major"
    [64 part (k), b*800 + j] group tiles during PSUM evacuation (strided
    dst APs); O-proj reads ctx back via strided rhs APs.
  - Softmax: scoresT = K_h @ Q_h^T computed directly ([t' part, t free]);
    no max subtraction (inputs are scale-0.02 gaussians, scores are O(0.3),
    exp cannot overflow; max-sub cancels exactly in exact arithmetic).
    1/rowsum is PE-broadcast and folded into the ctx evacuation multiply.
  - GRU: feature-major, weight-stationary Whh matmuls; xp precomputed for
    all steps and injected into the PSUM accumulation group via identity
    matmuls (so the group is well-formed: start on the first preload).
"""

import numpy as np

# ---------------- model dims ----------------
B, S, D, NH, HFF, VOCAB = 128, 100, 512, 8, 2048, 50000
DH = D // NH
GH, GL, NCLS = 256, 2, 10
NE = 3
NCORES = 8
BL = B // NCORES          # 16 batch per core
T = BL * S                # 1600 tokens per core
TN = 400                  # token tile (N dim) = 4 batches
NT = T // TN              # 4 batch-groups
J = NH * S                # 800
SCALE = DH ** -0.5
F32 = np.float32


# ---------------- host packing helpers ----------------
def _f32(a):
    return np.ascontiguousarray(np.asarray(a), dtype=np.float32)


def _pack_lhsT(w):
    """w [K, M] -> [128, nk*nm*128]; chunk (kc, mt) at col (kc*nm+mt)*128."""
    K, M = w.shape
    nk, nm = K // 128, M // 128
    out = np.empty((128, nk * nm * 128), dtype=F32)
    for kc in range(nk):
        for mt in range(nm):
            out[:, (kc * nm + mt) * 128:(kc * nm + mt + 1) * 128] = \
                w[kc * 128:(kc + 1) * 128, mt * 128:(mt + 1) * 128]
    return out


def _pack_qkv(w):
    """w [512, 512] -> [128, 4*8*64]; chunk (kc, dc) at col (kc*8+dc)*64."""
    out = np.empty((128, 4 * 8 * 64), dtype=F32)
    for kc in range(4):
        for dc in range(8):
            out[:, (kc * 8 + dc) * 64:(kc * 8 + dc + 1) * 64] = \
                w[kc * 128:(kc + 1) * 128, dc * 64:(dc + 1) * 64]
    return out


def _pack_wo(w):
    """w [512, 512] -> [64, 8*4*128]; chunk (c8, mt) at col (c8*4+mt)*128."""
    out = np.empty((64, 8 * 4 * 128), dtype=F32)
    for c8 in range(8):
        for mt in range(4):
            out[:, (c8 * 4 + mt) * 128:(c8 * 4 + mt + 1) * 128] = \
                w[c8 * 64:(c8 + 1) * 64, mt * 128:(mt + 1) * 128]
    return out


class _Cols:
    def __init__(self):
        self.blocks = []
        self.n = 0

    def add(self, block):
        block = np.asarray(block, np.float32)
        if block.ndim == 1:
            block = block.reshape(-1, 1)
        if block.shape[0] < 128:
            block = np.concatenate(
                [block, np.zeros((128 - block.shape[0], block.shape[1]),
                                 np.float32)], 0)
        off = self.n
        self.blocks.append(block)
        self.n += block.shape[1]
        return off

    def data(self):
        return np.concatenate(self.blocks, axis=1)


def prepare_host_inputs(x, emb, Wq, bq, Wk, bk, Wv, bv, Wo, bo, g1, be1,
                        W1, b1, W2, b2, g2, be2, gru_Wih, gru_Whh,
                        gru_bih, gru_bhh, fc_W, fc_b):
    emb = _f32(emb)
    Wq, Wk, Wv, Wo = _f32(Wq), _f32(Wk), _f32(Wv), _f32(Wo)
    W1, W2 = _f32(W1), _f32(W2)
    g1, be1, g2, be2 = _f32(g1), _f32(be1), _f32(g2), _f32(be2)
    bq, bk, bv, bo, b1, b2 = map(_f32, (bq, bk, bv, bo, b1, b2))
    gru_Wih, gru_Whh = _f32(gru_Wih), _f32(gru_Whh)
    gru_bih, gru_bhh = _f32(gru_bih), _f32(gru_bhh)
    fc_W, fc_b = _f32(fc_W), _f32(fc_b)

    wq = np.concatenate([_pack_qkv(Wq[l]) for l in range(NE)], axis=1)
    wk = np.concatenate([_pack_qkv(Wk[l]) for l in range(NE)], axis=1)
    wv = np.concatenate([_pack_qkv(Wv[l]) for l in range(NE)], axis=1)
    wo = np.concatenate([_pack_wo(Wo[l]) for l in range(NE)], axis=1)
    w1 = np.concatenate([_pack_lhsT(W1[l]) for l in range(NE)], axis=1)
    w2 = np.concatenate([_pack_lhsT(W2[l]) for l in range(NE)], axis=1)

    wih_blocks = []
    for l in range(GL):
        for d in range(2):
            WT = np.ascontiguousarray(gru_Wih[l, d].T)
            for kcu in range(4):
                r0 = kcu * 128 if l == 0 else (kcu % 2) * 256 + (kcu // 2) * 128
                for mtg in range(6):
                    wih_blocks.append(np.ascontiguousarray(
                        WT[r0:r0 + 128, mtg * 128:(mtg + 1) * 128]))
    wih = np.concatenate(wih_blocks, axis=1)

    whh_blocks = []
    for l in range(GL):
        for d in range(2):
            WT = np.ascontiguousarray(gru_Whh[l, d].T)
            for kc in range(2):
                for mtg in range(6):
                    whh_blocks.append(np.ascontiguousarray(
                        WT[kc * 128:(kc + 1) * 128,
                           mtg * 128:(mtg + 1) * 128]))
    whh = np.concatenate(whh_blocks, axis=1)

    wfc = fc_W.reshape(8, 128, NCLS).transpose(1, 0, 2).reshape(128, 8 * NCLS)
    wfc = np.ascontiguousarray(wfc)

    cc = _Cols()
    off = {}
    for name, bb in (('bq', bq), ('bk', bk), ('bv', bv)):
        off[name] = cc.add(np.stack(
            [bb[l, dc * 64:(dc + 1) * 64]
             for l in range(NE) for dc in range(8)], 1))
    off['bo'] = cc.add(np.stack(
        [bo[l, m * 128:(m + 1) * 128] for l in range(NE) for m in range(4)], 1))
    off['b1'] = cc.add(np.stack(
        [b1[l, m * 128:(m + 1) * 128] for l in range(NE) for m in range(16)], 1))
    off['b2'] = cc.add(np.stack(
        [b2[l, m * 128:(m + 1) * 128] for l in range(NE) for m in range(4)], 1))
    off['g'] = cc.add(np.stack(
        [(g1 if ln == 0 else g2)[l, m * 128:(m + 1) * 128]
         for l in range(NE) for ln in range(2) for m in range(4)], 1))
    off['be'] = cc.add(np.stack(
        [(be1 if ln == 0 else be2)[l, m * 128:(m + 1) * 128]
         for l in range(NE) for ln in range(2) for m in range(4)], 1))
    xpb = []
    for l in range(GL):
        for d in range(2):
            for mtg in range(6):
                g_, c_ = mtg // 2, mtg % 2
                r0 = g_ * 256 + c_ * 128
                v = gru_bih[l, d, r0:r0 + 128].copy()
                if g_ < 2:
                    v += gru_bhh[l, d, r0:r0 + 128]
                xpb.append(v)
    off['xpb'] = cc.add(np.stack(xpb, 1))
    # bhh_n preload [128, l*64 + d*32 + c*16 + b]
    bn = np.zeros((128, GL * 64), np.float32)
    for l in range(GL):
        for d in range(2):
            for c_ in range(2):
                v = gru_bhh[l, d, 2 * GH + c_ * 128: 2 * GH + (c_ + 1) * 128]
                for b_ in range(16):
                    bn[:, l * 64 + d * 32 + c_ * 16 + b_] = v
    off['bhhn'] = cc.add(bn)
    off['eps'] = cc.add(np.full((128, 1), 1e-5, np.float32))
    off['_w'] = cc.n
    cdata = cc.data()

    eyec = np.zeros((128, 64), dtype=F32)
    eyec[:64, :] = np.eye(64, dtype=F32)
    eye128 = np.eye(128, dtype=F32)
    fcbr = fc_b.reshape(1, NCLS).astype(F32)
    bhhn_b = np.ascontiguousarray(cdata[:, off['bhhn']:off['bhhn'] + GL * 64])

    shared = dict(wq=wq, wk=wk, wv=wv, wo=wo, w1=w1, w2=w2,
                  wih=wih, whh=whh, wfc=wfc,
                  cconst=cdata, eyec=eyec, eye128=eye128, bhhn_b=bhhn_b,
                  fcbr=fcbr)

    x = np.asarray(x)
    in_maps = []
    for core in range(NCORES):
        xl = x[core * BL:(core + 1) * BL]
        e = emb[xl]                                   # [16, 100, 512]
        xt = e.transpose(2, 0, 1).reshape(D, T)
        x0 = xt.reshape(4, 128, T).transpose(1, 0, 2).reshape(128, 4 * T)
        m = dict(shared)
        m['x0t'] = np.ascontiguousarray(x0)
        in_maps.append(m)
    return off, in_maps


# ---------------- device program ----------------
_CACHE = {}


def build_program(off):
    import sys
    if '/opt/trn_rl_repo' not in sys.path:
        sys.path.insert(0, '/opt/trn_rl_repo')
    import concourse.tile as tile
    import concourse.mybir as mybir
    from concourse import bacc
    from contextlib import ExitStack

    dt = mybir.dt
    F = dt.float32
    AF = mybir.ActivationFunctionType
    ALU = mybir.AluOpType
    AX = mybir.AxisListType

    nc = bacc.Bacc("TRN2", target_bir_lowering=False, debug=False,
                   num_devices=NCORES)

    def din(name, shape):
        return nc.dram_tensor(name, shape, F, kind="ExternalInput").ap()

    x0t = din('x0t', [128, 4 * T])
    wq_d = din('wq', [128, NE * 2048])
    wk_d = din('wk', [128, NE * 2048])
    wv_d = din('wv', [128, NE * 2048])
    wo_d = din('wo', [64, NE * 4096])
    w1_d = din('w1', [128, NE * 8192])
    w2_d = din('w2', [128, NE * 8192])
    wih_d = din('wih', [128, GL * 6144])
    whh_d = din('whh', [128, GL * 3072])
    wfc_d = din('wfc', [128, 8 * NCLS])
    cconst_d = din('cconst', [128, off['_w']])
    eyec_d = din('eyec', [128, 64])
    eye128_d = din('eye128', [128, 128])
    bhhn_d = din('bhhn_b', [128, GL * 64])
    fcbr_d = din('fcbr', [1, NCLS])
    out_d = nc.dram_tensor('out', [BL, NCLS], F,
                           kind="ExternalOutput").ap()

    def ln_pass(tc, src, dst, cst, ones128, onesr128, lp, pp, pb, l, ln):
        """LN over feature dim: dst = (src - mean)/std * g + be (per token)."""
        sq = lp.tile([128, 4 * T], F, tag="sq", bufs=1)
        nc.gpsimd.tensor_mul(sq[:], src[:], src[:])
        srow = lp.tile([1, T], F, tag="srow", bufs=1)
        qrow = lp.tile([1, T], F, tag="qrow", bufs=1)
        for nt in range(NT):
            psa = pp.tile([1, TN], F, tag="a")
            for kc in range(4):
                nc.tensor.matmul(
                    psa[:], ones128[:],
                    src[:, kc * T + nt * TN:kc * T + nt * TN + TN],
                    start=(kc == 0), stop=(kc == 3))
            nc.vector.tensor_copy(srow[:, nt * TN:(nt + 1) * TN], psa[:])
            psb = pp.tile([1, TN], F, tag="b")
            for kc in range(4):
                nc.tensor.matmul(
                    psb[:], ones128[:],
                    sq[:, kc * T + nt * TN:kc * T + nt * TN + TN],
                    start=(kc == 0), stop=(kc == 3))
            nc.vector.tensor_copy(qrow[:, nt * TN:(nt + 1) * TN], psb[:])
        mean = lp.tile([1, T], F, tag="mean", bufs=1)
        nc.vector.tensor_scalar_mul(mean[:], srow[:], 1.0 / D)
        msq = lp.tile([1, T], F, tag="rowtmp", bufs=2)
        nc.vector.tensor_mul(msq[:], mean[:], mean[:])
        var = lp.tile([1, T], F, tag="rowtmp", bufs=2)
        nc.vector.scalar_tensor_tensor(var[:], qrow[:], 1.0 / D, msq[:],
                                       ALU.mult, ALU.subtract)
        sd = lp.tile([1, T], F, tag="rowtmp", bufs=2)
        nc.scalar.activation(sd[:], var[:], AF.Sqrt,
                             bias=cst[0:1, off['eps']:off['eps'] + 1])
        rstd = lp.tile([1, T], F, tag="rstd", bufs=1)
        with nc.allow_low_precision(reason="LN 1/std"):
            nc.vector.reciprocal(rstd[:], sd[:])
        for nt in range(NT):
            psm = pb.tile([128, TN], F, tag="m")
            nc.tensor.matmul(psm[:], onesr128[:],
                             mean[:, nt * TN:(nt + 1) * TN],
                             start=True, stop=True)
            psr = pb.tile([128, TN], F, tag="r")
            nc.tensor.matmul(psr[:], onesr128[:],
                             rstd[:, nt * TN:(nt + 1) * TN],
                             start=True, stop=True)
            mB = lp.tile([128, TN], F, tag="mB")
            nc.vector.tensor_copy(mB[:], psm[:])
            rB = lp.tile([128, TN], F, tag="rB")
            nc.vector.tensor_copy(rB[:], psr[:])
            for mt in range(4):
                sl = slice(mt * T + nt * TN, mt * T + nt * TN + TN)
                t1 = lp.tile([128, TN], F, tag="t1")
                nc.vector.tensor_sub(t1[:], src[:, sl], mB[:])
                t2 = lp.tile([128, TN], F, tag="t2")
                nc.gpsimd.tensor_mul(t2[:], t1[:], rB[:])
                nc.vector.tensor_scalar(
                    dst[:, sl], t2[:],
                    cst[:, off['g'] + (l * 2 + ln) * 4 + mt:
                        off['g'] + (l * 2 + ln) * 4 + mt + 1],
                    cst[:, off['be'] + (l * 2 + ln) * 4 + mt:
                        off['be'] + (l * 2 + ln) * 4 + mt + 1],
                    ALU.mult, ALU.add)

    with tile.TileContext(nc) as tc:
        with ExitStack() as es:
            pers = es.enter_context(tc.tile_pool(name="pers", bufs=1))
            A = pers.tile([128, 4 * T], F, tag="A")
            cst = pers.tile([128, off['_w']], F, tag="cst")
            eye_s = pers.tile([128, 64], F, tag="eye")
            eye128_s = pers.tile([128, 128], F, tag="eye128")
            bhhn_s = pers.tile([128, GL * 64], F, tag="bhhn")
            fcb_s = pers.tile([1, NCLS], F, tag="fcb")
            ones128 = pers.tile([128, 1], F, tag="o128")
            onesr128 = pers.tile([1, 128], F, tag="or128")
            ones100 = pers.tile([100, 1], F, tag="o100")
            onesr64 = pers.tile([1, 64], F, tag="or64")
            onesr16 = pers.tile([1, 16], F, tag="or16")
            hz = pers.tile([128, 32], F, tag="hz")

            nc.sync.dma_start(A[:], x0t[:])
            nc.sync.dma_start(cst[:], cconst_d[:])
            nc.sync.dma_start(eye_s[:], eyec_d[:])
            nc.sync.dma_start(eye128_s[:], eye128_d[:])
            nc.sync.dma_start(bhhn_s[:], bhhn_d[:])
            nc.sync.dma_start(fcb_s[:], fcbr_d[:])
            nc.vector.memset(ones128[:], 1.0)
            nc.vector.memset(onesr128[:], 1.0)
            nc.vector.memset(ones100[:], 1.0)
            nc.vector.memset(onesr64[:], 1.0)
            nc.vector.memset(onesr16[:], 1.0)
            nc.vector.memset(hz[:], 0.0)

            def ccol(name, idx, p=128):
                return cst[0:p, off[name] + idx: off[name] + idx + 1]

            # ================= encoder =================
            with tc.tile_pool(name="encp", bufs=1) as ep:
                Bt = ep.tile([128, 4 * T], F, tag="B")
                for l in range(NE):
                    with tc.tile_pool(name=f"wqkv{l}", bufs=1) as wp, \
                         tc.tile_pool(name=f"jg{l}", bufs=2) as jg, \
                         tc.tile_pool(name=f"att{l}", bufs=3) as ap_, \
                         tc.tile_pool(name=f"psq{l}", bufs=4,
                                      space="PSUM") as pq, \
                         tc.tile_pool(name=f"psS{l}", bufs=2,
                                      space="PSUM") as pS, \
                         tc.tile_pool(name=f"psRB{l}", bufs=2,
                                      space="PSUM") as pR, \
                         tc.tile_pool(name=f"psV{l}", bufs=2,
                                      space="PSUM") as pV:
                        wq_s = wp.tile([128, 2048], F, tag="wq")
                        wk_s = wp.tile([128, 2048], F, tag="wk")
                        wv_s = wp.tile([128, 2048], F, tag="wv")
                        wo_s = wp.tile([64, 4096], F, tag="wo")
                        nc.sync.dma_start(wq_s[:],
                                          wq_d[:, l * 2048:(l + 1) * 2048])
                        nc.sync.dma_start(wk_s[:],
                                          wk_d[:, l * 2048:(l + 1) * 2048])
                        nc.sync.dma_start(wv_s[:],
                                          wv_d[:, l * 2048:(l + 1) * 2048])
                        nc.sync.dma_start(wo_s[:],
                                          wo_d[:, l * 4096:(l + 1) * 4096])
                        for bg in range(NT):
                            qjg = jg.tile([64, 4 * J], F, tag="qj")
                            kjg = jg.tile([64, 4 * J], F, tag="kj")
                            vjg = jg.tile([64, 4 * J], F, tag="vj")
                            cjg = jg.tile([64, 4 * J], F, tag="cj", bufs=1)
                            for wsb, dst, bname in ((wq_s, qjg, 'bq'),
                                                    (wk_s, kjg, 'bk'),
                                                    (wv_s, vjg, 'bv')):
                                dstr = dst[:, :].rearrange(
                                    "p (b s e) -> p b s e", b=4, s=S)
                                for dc in range(8):
                                    ps = pq.tile([64, TN], F, tag="ps", bufs=2,
                                                 padded_shape=[128, TN])
                                    for kc in range(4):
                                        nc.tensor.matmul(
                                            ps[:],
                                            wsb[:, (kc * 8 + dc) * 64:
                                                (kc * 8 + dc + 1) * 64],
                                            A[:, kc * T + bg * TN:
                                              kc * T + bg * TN + TN],
                                            start=(kc == 0), stop=(kc == 3))
                                    nc.vector.tensor_scalar_add(
                                        dstr[:, :, :, dc],
                                        ps[:].rearrange("p (b s) -> p b s",
                                                        b=4),
                                        ccol(bname, l * 8 + dc, p=64))
                            for br in range(4):
                                for hg in range(2):
                                    base = br * J + hg * 400
                                    psS = pS.tile([100, 400], F, tag="s", bufs=2)
                                    for hh in range(4):
                                        h0 = base + hh * 100
                                        nc.tensor.matmul(
                                            psS[:, hh * 100:(hh + 1) * 100],
                                            kjg[:, h0:h0 + 100],
                                            qjg[:, h0:h0 + 100],
                                            start=(hh == 0), stop=(hh == 3))
                                    expT = ap_.tile([100, 400], F, tag="e")
                                    nc.scalar.activation(expT[:], psS[:],
                                                         AF.Exp, scale=SCALE)
                                    psR = pR.tile([1, 400], F, tag="r", bufs=1)
                                    nc.tensor.matmul(psR[:], ones100[:],
                                                     expT[:],
                                                     start=True, stop=True)
                                    rsum = ap_.tile([1, 400], F, tag="rs")
                                    with nc.allow_low_precision(
                                            reason="softmax 1/sum"):
                                        nc.vector.reciprocal(rsum[:], psR[:])
                                    psB = pR.tile([64, 400], F, tag="b", bufs=1)
                                    nc.tensor.matmul(psB[:], onesr64[:],
                                                     rsum[:],
                                                     start=True, stop=True)
                                    bB = ap_.tile([64, 400], F, tag="bB", bufs=2)
                                    nc.vector.tensor_copy(bB[:], psB[:])
                                    psV = pV.tile([100, 256], F, tag="v", bufs=1)
                                    for hh in range(4):
                                        h0 = base + hh * 100
                                        nc.tensor.transpose(
                                            psV[:, hh * 64:(hh + 1) * 64],
                                            vjg[:, h0:h0 + 100],
                                            eye_s[0:64, :])
                                    vtok = ap_.tile([100, 256], F, tag="vt")
                                    nc.vector.tensor_copy(vtok[:], psV[:])
                                    psC = pS.tile([64, 400], F, tag="c", bufs=1)
                                    for hh in range(4):
                                        nc.tensor.matmul(
                                            psC[:, hh * 100:(hh + 1) * 100],
                                            vtok[:, hh * 64:(hh + 1) * 64],
                                            expT[:, hh * 100:(hh + 1) * 100],
                                            start=(hh == 0), stop=(hh == 3))
                                    nc.vector.tensor_mul(
                                        cjg[:, base:base + 400],
                                        psC[:], bB[:])
                            ctx_r = cjg[:, :].rearrange(
                                "p (b s e) -> p b s e", b=4, s=S)
                            for mt in range(4):
                                ps = pq.tile([128, TN], F, tag="ps", bufs=2)
                                for c8 in range(8):
                                    nc.tensor.matmul(
                                        ps[:],
                                        wo_s[:, (c8 * 4 + mt) * 128:
                                             (c8 * 4 + mt + 1) * 128],
                                        ctx_r[:, :, :, c8],
                                        start=(c8 == 0), stop=(c8 == 7))
                                sl = slice(mt * T + bg * TN,
                                           mt * T + bg * TN + TN)
                                nc.vector.scalar_tensor_tensor(
                                    Bt[:, sl], ps[:], ccol('bo', l * 4 + mt),
                                    A[:, sl], ALU.add, ALU.add)

                    with tc.tile_pool(name=f"ln0_{l}", bufs=2) as lp, \
                         tc.tile_pool(name=f"lp0s{l}", bufs=2,
                                      space="PSUM") as pp, \
                         tc.tile_pool(name=f"lp0b{l}", bufs=2,
                                      space="PSUM") as pb:
                        ln_pass(tc, Bt, A, cst, ones128, onesr128,
                                lp, pp, pb, l, 0)

                    with tc.tile_pool(name=f"wff{l}", bufs=1) as wp3, \
                         tc.tile_pool(name=f"ffh{l}", bufs=2) as fh, \
                         tc.tile_pool(name=f"psF{l}", bufs=4,
                                      space="PSUM") as pF:
                        w1_s = wp3.tile([128, 8192], F, tag="w1")
                        w2_s = wp3.tile([128, 8192], F, tag="w2")
                        nc.sync.dma_start(w1_s[:],
                                          w1_d[:, l * 8192:(l + 1) * 8192])
                        nc.sync.dma_start(w2_s[:],
                                          w2_d[:, l * 8192:(l + 1) * 8192])
                        for nt in range(NT):
                            ffh = fh.tile([128, 16 * TN], F, tag="h")
                            for mth in range(16):
                                ps = pF.tile([128, TN], F, tag="p1")
                                for kc in range(4):
                                    nc.tensor.matmul(
                                        ps[:],
                                        w1_s[:, (kc * 16 + mth) * 128:
                                             (kc * 16 + mth + 1) * 128],
                                        A[:, kc * T + nt * TN:
                                          kc * T + nt * TN + TN],
                                        start=(kc == 0), stop=(kc == 3))
                                nc.scalar.activation(
                                    ffh[:, mth * TN:(mth + 1) * TN], ps[:],
                                    AF.Relu, bias=ccol('b1', l * 16 + mth))
                            for mt in range(4):
                                ps2 = pF.tile([128, TN], F, tag="p2")
                                for kc2 in range(16):
                                    nc.tensor.matmul(
                                        ps2[:],
                                        w2_s[:, (kc2 * 4 + mt) * 128:
                                             (kc2 * 4 + mt + 1) * 128],
                                        ffh[:, kc2 * TN:(kc2 + 1) * TN],
                                        start=(kc2 == 0), stop=(kc2 == 15))
                                sl = slice(mt * T + nt * TN,
                                           mt * T + nt * TN + TN)
                                nc.vector.scalar_tensor_tensor(
                                    Bt[:, sl], ps2[:],
                                    ccol('b2', l * 4 + mt),
                                    A[:, sl], ALU.add, ALU.add)

                    with tc.tile_pool(name=f"ln1_{l}", bufs=2) as lp, \
                         tc.tile_pool(name=f"lp1s{l}", bufs=2,
                                      space="PSUM") as pp, \
                         tc.tile_pool(name=f"lp1b{l}", bufs=2,
                                      space="PSUM") as pb:
                        ln_pass(tc, Bt, A, cst, ones128, onesr128,
                                lp, pp, pb, l, 1)

            # ================= GRU =================
            with tc.tile_pool(name="gru", bufs=1) as gp:
                whh_s = gp.tile([128, GL * 3072], F, tag="whh")
                nc.sync.dma_start(whh_s[:], whh_d[:])
                xp = gp.tile([128, S * 192], F, tag="xp")
                Y = gp.tile([128, 2 * 2 * BL * S], F, tag="y")
                xp_r = xp[:, :].rearrange("p (s d g c b) -> p s d g c b",
                                          s=S, d=2, g=3, c=2)
                xp_q = xp[:, :].rearrange("p (s q) -> p q s", q=192)
                Yor = Y[:, :].rearrange(
                    "p (c2 d2 b s) -> p c2 d2 b s", c2=2, d2=2, b=BL)

                for l in range(GL):
                    with tc.tile_pool(name=f"wih{l}", bufs=1) as wp4, \
                         tc.tile_pool(name=f"psX{l}", bufs=4,
                                      space="PSUM") as pX:
                        wih_s = wp4.tile([128, 6144], F, tag="wih")
                        nc.sync.dma_start(
                            wih_s[:], wih_d[:, l * 6144:(l + 1) * 6144])
                        for d in range(2):
                            for mtg in range(6):
                                g_, c_ = mtg // 2, mtg % 2
                                q0 = d * 96 + g_ * 32 + c_ * 16
                                for nt in range(NT):
                                    ps = pX.tile([128, TN], F, tag="ps")
                                    for kcu in range(4):
                                        if l == 0:
                                            rhs = A[:, kcu * T + nt * TN:
                                                    kcu * T + nt * TN + TN]
                                        else:
                                            ci, di = kcu // 2, kcu % 2
                                            b0 = ci * 3200 + di * 1600
                                            rhs = Y[:, b0 + nt * TN:
                                                    b0 + nt * TN + TN]
                                        wcol = ((d * 4 + kcu) * 6 + mtg) * 128
                                        nc.tensor.matmul(
                                            ps[:],
                                            wih_s[:, wcol:wcol + 128],
                                            rhs, start=(kcu == 0),
                                            stop=(kcu == 3))
                                    nc.vector.tensor_scalar_add(
                                        xp_q[:, q0 + nt * 4:q0 + nt * 4 + 4,
                                             :],
                                        ps[:].rearrange("p (b s) -> p b s",
                                                        b=4),
                                        ccol('xpb', (l * 2 + d) * 6 + mtg))

                    with tc.tile_pool(name=f"sc{l}", bufs=3) as sp, \
                         tc.tile_pool(name=f"psg{l}", bufs=3,
                                      space="PSUM") as pG:
                        for t in range(S):
                            ps = pG.tile([128, 192], F, tag="g")
                            psr = ps[:, :].rearrange(
                                "p (d g c b) -> p d g c b", d=2, g=3, c=2)
                            last_pre = (t == 0)
                            for d in range(2):
                                td = t if d == 0 else S - 1 - t
                                nc.tensor.matmul(
                                    ps[:, d * 96:d * 96 + 64],
                                    eye128_s[:],
                                    xp_r[:, td, d, 0:2, :, :],
                                    start=(d == 0), stop=False)
                                nc.tensor.matmul(
                                    ps[:, d * 96 + 64:d * 96 + 96],
                                    eye128_s[:],
                                    bhhn_s[:, l * 64 + d * 32:
                                           l * 64 + d * 32 + 32],
                                    start=False,
                                    stop=(last_pre and d == 1))
                            if t > 0:
                                for d in range(2):
                                    tp = (t - 1) if d == 0 else (S - t)
                                    for mtg in range(6):
                                        g_, c_ = mtg // 2, mtg % 2
                                        q0 = d * 96 + g_ * 32 + c_ * 16
                                        for kc in range(2):
                                            wcol = (((l * 2 + d) * 2 + kc)
                                                    * 6 + mtg) * 128
                                            nc.tensor.matmul(
                                                ps[:, q0:q0 + 16],
                                                whh_s[:, wcol:wcol + 128],
                                                Yor[:, kc, d, :, tp],
                                                start=False,
                                                stop=(d == 1 and mtg == 5
                                                      and kc == 1))
                            rz = sp.tile([128, 128], F, tag="rz")
                            rzr = rz[:, :].rearrange(
                                "p (d g c b) -> p d g c b", d=2, g=2, c=2)
                            nc.scalar.activation(
                                rzr[:, :, :, :, :],
                                psr[:, :, 0:2, :, :], AF.Sigmoid)
                            tn = sp.tile([128, 64], F, tag="tn")
                            tnr = tn[:, :].rearrange(
                                "p (d c b) -> p d c b", d=2, c=2)
                            nc.vector.scalar_tensor_tensor(
                                tnr, psr[:, :, 2, :, :], 0.0,
                                rzr[:, :, 0, :, :],
                                ALU.bypass, ALU.mult)
                            tn2 = sp.tile([128, 64], F, tag="tn2")
                            t2r = tn2[:, :].rearrange(
                                "p (d c b) -> p d c b", d=2, c=2)
                            for d in range(2):
                                td = t if d == 0 else S - 1 - t
                                nc.vector.tensor_add(
                                    t2r[:, d, :, :], tnr[:, d, :, :],
                                    xp_r[:, td, d, 2, :, :])
                            nn_ = sp.tile([128, 64], F, tag="nn")
                            nc.scalar.activation(nn_[:], tn2[:], AF.Tanh)
                            nnr = nn_[:, :].rearrange(
                                "p (d c b) -> p d c b", d=2, c=2)
                            for d in range(2):
                                td = t if d == 0 else S - 1 - t
                                tp = (t - 1) if d == 0 else (S - t)
                                n_sl = nnr[:, d, :, :]
                                if t == 0:
                                    hprev = hz[:, :].rearrange(
                                        "p (c b) -> p c b", c=2)
                                else:
                                    hprev = Yor[:, :, d, :, tp]
                                e1 = sp.tile([128, 32], F, tag="e1")
                                e1r = e1[:, :].rearrange("p (c b) -> p c b",
                                                         c=2)
                                nc.vector.tensor_sub(e1r, hprev, n_sl)
                                e2 = sp.tile([128, 32], F, tag="e2")
                                e2r = e2[:, :].rearrange("p (c b) -> p c b",
                                                         c=2)
                                nc.vector.scalar_tensor_tensor(
                                    e2r, e1r, 0.0,
                                    rzr[:, d, 1, :, :],
                                    ALU.bypass, ALU.mult)
                                nc.vector.tensor_add(
                                    Yor[:, :, d, :, td], e2r, n_sl)

                # ============ pooling + FC ============
                with tc.tile_pool(name="fin", bufs=1) as fp, \
                     tc.tile_pool(name="psf", bufs=2, space="PSUM") as pFc:
                    wfc_s = fp.tile([128, 8 * NCLS], F, tag="wfc")
                    nc.sync.dma_start(wfc_s[:], wfc_d[:])
                    pooled = fp.tile([128, 8 * 16], F, tag="pool")
                    for kc in range(4):
                        pe = fp.tile([128, 16], F, tag="pe")
                        nc.vector.tensor_reduce(
                            out=pe[:],
                            in_=A[:, kc * T:(kc + 1) * T].rearrange(
                                "p (b s) -> p b s", b=BL),
                            op=ALU.max, axis=AX.X)
                        nc.scalar.activation(
                            pooled[:, kc * 16:(kc + 1) * 16], pe[:], AF.Relu)
                    for c_ in range(2):
                        for d in range(2):
                            pg_ = fp.tile([128, 16], F, tag="pg")
                            nc.vector.tensor_reduce(
                                out=pg_[:], in_=Yor[:, c_, d, :, :],
                                op=ALU.max, axis=AX.X)
                            nc.scalar.activation(
                                pooled[:, (4 + d * 2 + c_) * 16:
                                       (4 + d * 2 + c_ + 1) * 16],
                                pg_[:], AF.Relu)
                    psf = pFc.tile([BL, NCLS], F, tag="f")
                    for ch in range(8):
                        nc.tensor.matmul(
                            psf[:], pooled[:, ch * 16:(ch + 1) * 16],
                            wfc_s[:, ch * NCLS:(ch + 1) * NCLS],
                            start=(ch == 0), stop=False)
                    nc.tensor.matmul(psf[:], onesr16[:], fcb_s[:],
                                     start=False, stop=True)
                    ores = fp.tile([BL, NCLS], F, tag="or")
                    nc.vector.tensor_copy(ores[:], psf[:])
                    nc.sync.dma_start(out_d[:], ores[:])

    nc.compile()
    return nc


# ---------------- top-level entry ----------------
def kernel(x, x1, emb, Wq, bq, Wk, bk, Wv, bv, Wo, bo, g1, be1, W1, b1,
           W2, b2, g2, be2, gru_Wih, gru_Whh, gru_bih, gru_bhh, fc_W, fc_b):
    off, in_maps = prepare_host_inputs(
        x, emb, Wq, bq, Wk, bk, Wv, bv, Wo, bo, g1, be1, W1, b1, W2, b2,
        g2, be2, gru_Wih, gru_Whh, gru_bih, gru_bhh, fc_W, fc_b)
    if 'prog' not in _CACHE:
        _CACHE['prog'] = build_program(off)
    nc = _CACHE['prog']
    import sys
    if '/opt/trn_rl_repo' not in sys.path:
        sys.path.insert(0, '/opt/trn_rl_repo')
    from concourse.bass_utils import run_bass_kernel_spmd
    res = run_bass_kernel_spmd(nc, in_maps, list(range(NCORES)))
    outs = [res.results[i]['out'] for i in range(NCORES)]
    return np.concatenate(outs, axis=0).astype(np.float32)
